# revision 36
# baseline (speedup 1.0000x reference)
"""GraphSAGE (2x SAGEConv mean-aggr + MLP decoder) on 8 Trainium2 NeuronCores.

v4 design (sim-trace driven):
- dst-node sharding, 12500/core padded to 12800 (4 quarters x 3200) so the
  h1 AllGather can run in 4 row-chunks overlapped with round-1 tail and
  round-2 bank-ordered gathers.
- Unified node numbering for both rounds: node v lives at row
  (v//12500)*12800 + quarter padding in BOTH the x table and the h1 table,
  so both rounds share identical gather indices and masks.
- WINDOWED one-hot masks: each 128-slot chunk (dst-sorted within one
  (sb,bank) segment) touches a ~56-wide dst window shared across cores, so
  the fp8 mask stores only [off_c, off_c+w_c) columns packed flat
  (12.6MB/round vs 57.5 full-width) and the PE matmul runs w_c cols
  instead of 256 (sim PE busy 432us -> 135us). The first chunk of every
  sb keeps the full 256 window and runs start=True, which doubles as the
  PSUM zero-init; later chunks accumulate into their column windows.
- Masks and idx split across the two HWDGE rings (SP + ACT) per sb parity.
- AllGather chunk q emitted mid-round-1 right after quarter q's h1tab_in
  rows are written, so the collective overlaps the round-1 tail on the
  in-order Pool queue.
- Mean: DVE multiply with host-replicated 1/deg; linears in T-orientation;
  round 1 relu -> SBUF-resident h1selfT -> PE transpose -> h1tab_in;
  decoder fused per superbatch, b2 folded into the decoder bias.
"""

import os

import numpy as np
import ml_dtypes

import concourse.bacc as bacc
import concourse.bass as bass
import concourse.mybir as mybir
import concourse.tile as tile
from concourse.bass_utils import run_bass_kernel_spmd
from concourse.library_config import mlp as mlp_lib

BF16 = ml_dtypes.bfloat16
FP8 = ml_dtypes.float8_e4m3fn

ABLATE = set(os.environ.get("K2_ABLATE", "").split(","))

N_CORES = 8
D = 128
P = 128
SB_NODES = 256
BANK = 25600
GATHER_CAP = 8192

SHARD = 12500
QCAP = 3200           # quarter capacity (multiple of 128)
SHARD_PAD = 4 * QCAP  # 12800
N_PAD = N_CORES * SHARD_PAD  # 102400
N_SB = SHARD_PAD // SB_NODES  # 50
N_BANKS = (N_PAD + BANK - 1) // BANK  # 4
GS = 5                # superbatches per gather group (one dma_gather per
                      # (group, bank) amortizes the ~1us SWDGE fixed cost)


def _pad_local(r):
    """local node index [0,12500) -> quarter-padded [0,12800)."""
    q = r // 3125
    return q * QCAP + (r - q * 3125)


def _unified_row(v):
    """global node id -> row in the unified padded table.

    Quarter-major: row = q*8*QCAP + core*QCAP + r_within_quarter, matching
    the layout the 4-chunk AllGather produces (chunk q = concat over cores
    of their quarter q), so AG chunk q fills exactly gather bank q."""
    c = v // SHARD
    r = v - c * SHARD
    q = r // 3125
    rq = r - q * 3125
    return q * (N_CORES * QCAP) + c * QCAP + rq


def _slot_meta(src_row, dst_pad, core_of_edge):
    """Group each core's edges by (sb, bank, dst); pad per-(sb,bank)
    segments to a common (max-over-cores, 128-aligned) budget.

    Returns per-core idx (int16 wrapped) + per-core fp8 one-hot masks
    [128, total_chunks, SB_NODES] + shared budgets/segment offsets."""
    sb = dst_pad // SB_NODES
    bank = src_row // BANK

    counts = np.zeros((N_CORES, N_SB, N_BANKS), dtype=np.int64)
    np.add.at(counts, (core_of_edge, sb, bank), 1)
    budgets = counts.max(axis=0)
    budgets = ((budgets + 127) // 128) * 128
    assert budgets.max() <= GATHER_CAP, budgets.max()

    # mask/slot-metadata order: sb-major (each sb's mask chunks contiguous)
    mask_seg_off = np.zeros((N_SB, N_BANKS), dtype=np.int64)
    flat = budgets.reshape(-1)
    mask_seg_off.reshape(-1)[1:] = np.cumsum(flat)[:-1]
    total_slots = int(flat.sum())
    n_chunks = total_slots // 128

    # gather order: pair-major, bank-major within pair (each (pair, bank)
    # is one contiguous dma_gather -> halves SWDGE call count)
    seg_off = np.zeros((N_SB, N_BANKS), dtype=np.int64)
    group_base = np.zeros(N_SB // GS + 1, dtype=np.int64)
    pos = 0
    for g in range(N_SB // GS):
        group_base[g] = pos
        for b in range(N_BANKS):
            for s in range(g * GS, (g + 1) * GS):
                seg_off[s, b] = pos
                pos += budgets[s, b]
    group_base[N_SB // GS] = pos
    assert pos == total_slots

    idx_cores, dw_cores = [], []
    for c in range(N_CORES):
        m = core_of_edge == c
        s_c, dp_c, sb_c, bk_c = (src_row[m], dst_pad[m], sb[m], bank[m])
        order = np.lexsort((dp_c, bk_c, sb_c))
        s_c, dp_c, sb_c, bk_c = (a[order] for a in (s_c, dp_c, sb_c, bk_c))

        idx_full = np.zeros(total_slots, dtype=np.int16)
        dstw_full = np.full(total_slots, -1, dtype=np.int64)
        cnt_c = np.zeros((N_SB, N_BANKS), dtype=np.int64)
        np.add.at(cnt_c, (sb_c, bk_c), 1)
        seg_start = np.zeros((N_SB, N_BANKS), dtype=np.int64)
        seg_start.reshape(-1)[1:] = np.cumsum(cnt_c.reshape(-1))[:-1]
        pos_in_seg = np.arange(len(s_c)) - seg_start[sb_c, bk_c]
        gslot = seg_off[sb_c, bk_c] + pos_in_seg
        mslot = mask_seg_off[sb_c, bk_c] + pos_in_seg
        idx_full[gslot] = (s_c - bk_c * BANK).astype(np.int16)
        dstw_full[mslot] = dp_c - sb_c * SB_NODES

        # idx wrap: slot i -> [i%16, i//16], replicated to 128 partitions
        w = idx_full.reshape(total_slots // 16, 16).T
        idx_cores.append(np.tile(w, (8, 1)).copy())
        dw_cores.append(dstw_full.reshape(n_chunks, 128))  # [chunks, 128]

    # Windowed masks: each chunk's slots (dst-sorted within one (sb,bank)
    # segment) touch a narrow dst window shared across cores. Store the
    # one-hot mask only over [off_c, off_c + w_c) columns, packed flat.
    # The FIRST chunk of every sb keeps the full SB_NODES window and runs
    # with start=True, which also zero-initializes the whole PSUM tile.
    all_dw = np.stack(dw_cores)                   # [cores, chunks, 128]
    vmask = all_dw >= 0
    lo = np.where(vmask, all_dw, SB_NODES).min(axis=(0, 2))   # [chunks]
    hi = np.where(vmask, all_dw, -1).max(axis=(0, 2))
    cw = np.where(hi >= lo, hi - lo + 1, 1)
    cw = np.minimum((cw + 15) // 16 * 16, SB_NODES)
    coff = np.minimum(np.maximum(lo, 0), SB_NODES - cw)
    chunk_sb = np.repeat(np.arange(N_SB), budgets.sum(axis=1) // 128)
    first = np.ones(n_chunks, dtype=bool)
    first[1:] = chunk_sb[1:] != chunk_sb[:-1]
    cw[first] = SB_NODES
    coff[first] = 0
    cflat = np.zeros(n_chunks + 1, dtype=np.int64)
    cflat[1:] = np.cumsum(cw)
    total_mask_cols = int(cflat[-1])

    mask_cores = []
    for c in range(N_CORES):
        dw = dw_cores[c]                          # [chunks, 128]
        mf = np.zeros((P, total_mask_cols), dtype=FP8)
        ch = np.repeat(np.arange(n_chunks), 128)
        pp = np.tile(np.arange(128), n_chunks)
        dwf = dw.reshape(-1)
        val = dwf >= 0
        mf[pp[val], cflat[ch[val]] + dwf[val] - coff[ch[val]]] = FP8(1.0)
        mask_cores.append(mf)

    return {
        "budgets": budgets, "seg_off": seg_off,
        "mask_seg_off": mask_seg_off, "group_base": group_base,
        "total_slots": total_slots,
        "n_chunks": n_chunks, "idx": idx_cores, "mask": mask_cores,
        "cw": cw, "coff": coff, "cflat": cflat,
        "total_mask_cols": total_mask_cols, "chunk_sb": chunk_sb,
    }


def prep(inputs):
    x = np.asarray(inputs["x"], dtype=np.float32)
    ei = np.asarray(inputs["edge_index"])
    n = x.shape[0]
    assert n == N_CORES * SHARD

    src = ei[0].astype(np.int64)
    dst = ei[1].astype(np.int64)
    src_row = _unified_row(src)
    core_of_edge = dst // SHARD
    dst_local = dst - core_of_edge * SHARD
    dst_pad = _pad_local(dst_local)

    sm = _slot_meta(src_row, dst_pad, core_of_edge)

    deg = np.bincount(dst, minlength=n).astype(np.float32)
    recip = (1.0 / np.maximum(deg, 1.0)).astype(np.float32)
    recip_pad = np.zeros((N_CORES, 1, SHARD_PAD), dtype=np.float32)
    x_selfT = np.zeros((N_CORES, D, SHARD_PAD), dtype=np.float32)
    for c in range(N_CORES):
        rl = _pad_local(np.arange(SHARD))
        recip_pad[c, 0, rl] = recip[c * SHARD:(c + 1) * SHARD]
        x_selfT[c, :, rl] = x[c * SHARD:(c + 1) * SHARD, :]

    x_pad = np.zeros((N_PAD, D), dtype=np.float32)
    x_pad[_unified_row(np.arange(n))] = x

    meta = {
        "sm": sm,
        "x_pad": x_pad.astype(BF16),
        "x_selfT": x_selfT.astype(BF16),
        "recip": recip_pad.astype(BF16),
        "W1_lT": np.asarray(inputs["W1_l"], np.float32).T.astype(BF16).copy(),
        "W1_rT": np.asarray(inputs["W1_r"], np.float32).T.astype(BF16).copy(),
        "W2_lT": np.asarray(inputs["W2_l"], np.float32).T.astype(BF16).copy(),
        "W2_rT": np.asarray(inputs["W2_r"], np.float32).T.astype(BF16).copy(),
        "W3T": np.asarray(inputs["W3"], np.float32).T.astype(BF16).copy(),
        "W4c": np.asarray(inputs["W4"], np.float32).reshape(2, 128).T
            .astype(BF16).copy(),
        "b1c": np.asarray(inputs["b1"], np.float32).reshape(-1, 1)
            .astype(BF16).copy(),
        # conv2 has no relu, so b2 folds exactly into the decoder bias:
        # relu(W3 @ (h2 + b2) + b3) = relu(W3 @ h2 + (b3 + W3 @ b2))
        "b3c": (np.asarray(inputs["b3"], np.float32)
                + np.asarray(inputs["W3"], np.float32)
                @ np.asarray(inputs["b2"], np.float32))
            .reshape(2, 128).T.astype(BF16).copy(),
        "b4": float(np.asarray(inputs["b4"]).reshape(-1)[0]),
        "ident": np.eye(P, dtype=BF16),
    }
    meta["recip_bc"] = np.broadcast_to(
        meta["recip"], (N_CORES, P, SHARD_PAD)).copy()
    return meta


WNAMES = ["W1_lT", "W1_rT", "W2_lT", "W2_rT", "W3T", "W4c",
          "b1c", "b3c", "ident"]


def build(meta):
    sm = meta["sm"]
    nc = bacc.Bacc("TRN2", target_bir_lowering=False, debug=False,
                   num_devices=N_CORES, num_swdge_queues=4)
    f32, bf16, fp8 = mybir.dt.float32, mybir.dt.bfloat16, mybir.dt.float8e4
    i16 = mybir.dt.int16

    x_tab = nc.dram_tensor("x_tab", [N_PAD, D], bf16, kind="ExternalInput")
    x_selfT_d = nc.dram_tensor("x_selfT", [D, SHARD_PAD], bf16,
                               kind="ExternalInput")
    recip_d = nc.dram_tensor("recip_bc", [P, SHARD_PAD], bf16,
                              kind="ExternalInput")
    idx_d = nc.dram_tensor("idx", list(sm["idx"][0].shape), i16,
                           kind="ExternalInput")
    mask_d = nc.dram_tensor("mask", [P, sm["total_mask_cols"]], fp8,
                            kind="ExternalInput")
    wt = {}
    for name in WNAMES:
        arr = meta[name]
        wt[name] = nc.dram_tensor(name, list(arr.shape), bf16,
                                  kind="ExternalInput")

    h1tab_in = nc.dram_tensor("h1tab_in", [SHARD_PAD, D], bf16)
    h1tab = nc.dram_tensor("h1tab", [N_PAD, D], bf16,
                           addr_space="Shared")
    out_shard = nc.dram_tensor("out_shard", [N_SB, SB_NODES], f32,
                               kind="ExternalOutput")

    budgets, seg_off = sm["budgets"], sm["seg_off"]
    mask_seg_off, group_base = sm["mask_seg_off"], sm["group_base"]
    cw, coff, cflat = sm["cw"], sm["coff"], sm["cflat"]

    with tile.TileContext(nc) as tc:
        with (
            tc.tile_pool(name="wp", bufs=1) as wp,
            tc.tile_pool(name="pp", bufs=2) as pp,
            tc.tile_pool(name="sp", bufs=6) as sp,
            tc.tile_pool(name="vp", bufs=4) as vp,
            tc.tile_pool(name="psA", bufs=2, space="PSUM") as psA,
            tc.tile_pool(name="psB", bufs=2, space="PSUM") as psB,
            tc.tile_pool(name="psM", bufs=2, space="PSUM") as psM,
            tc.tile_pool(name="psD", bufs=1, space="PSUM") as psD,
        ):
            with tc.tile_critical():
                nc.gpsimd.load_library(mlp_lib)

            consts = {}
            for name in WNAMES:
                t = wp.tile(list(meta[name].shape), bf16, tag=name)
                nc.sync.dma_start(t[:], wt[name][:])
                consts[name] = t
            recip_t = wp.tile([P, SHARD_PAD], bf16, tag="recip")
            nc.sync.dma_start(recip_t[:], recip_d[:])
            x_selfT_t = wp.tile([D, SHARD_PAD], bf16, tag="xselfT")
            nc.sync.dma_start(x_selfT_t[:], x_selfT_d[:])
            h1selfT_t = wp.tile([D, SHARD_PAD], bf16, tag="h1selfT")

            gq = [0]

            def emit_ag(q):
                if "ag" in ABLATE:
                    return
                nc.gpsimd.collective_compute(
                    "AllGather", mybir.AluOpType.bypass,
                    replica_groups=[list(range(N_CORES))],
                    ins=[h1tab_in[q * QCAP:(q + 1) * QCAP, :]],
                    outs=[h1tab[q * N_CORES * QCAP:
                                (q + 1) * N_CORES * QCAP, :]],
                )

            # emit AG chunk q as soon as the last h1tab_in row of quarter q
            # has been produced (quarter boundary 3200 = sb 12.5), so the
            # collective overlaps the round-1 tail on the in-order Pool queue
            AG_AFTER_SB = {12: 0, 24: 1, 37: 2, 49: 3}

            def emit_pair_gather(g, table):
                gbase = int(group_base[g])
                gslots = int(group_base[g + 1]) - gbase
                idx_t = pp.tile([P, gslots // 16], i16, tag="idx")
                iring = nc.scalar if g % 2 == 0 else nc.sync
                iring.dma_start(
                    idx_t[:],
                    idx_d[:, gbase // 16: (gbase + gslots) // 16])
                gat = pp.tile([P, gslots // 128, D], bf16, tag="gat")
                for b in range(N_BANKS):
                    nb = sum(int(budgets[s, b])
                             for s in range(g * GS, (g + 1) * GS))
                    if nb == 0:
                        continue
                    off = int(seg_off[g * GS, b]) - gbase
                    lo = b * BANK
                    hi = min(N_PAD, (b + 1) * BANK)
                    nc.gpsimd.dma_gather(
                        gat[:, off // 128: (off + nb) // 128, :],
                        table[lo:hi, :],
                        idx_t[:, off // 16: (off + nb) // 16],
                        num_idxs=nb, num_idxs_reg=nb, elem_size=D,
                        single_packet=False, queue_num=(b + g) % 4,
                    )
                    gq[0] += 1
                return gat, gbase

            def emit_round(rnd, table, wl, wr, brow):
                selfT = x_selfT_t if rnd == 0 else h1selfT_t
                for sb in range(N_SB):
                    if sb % GS == 0:
                        gat, gbase = emit_pair_gather(sb // GS, table)
                    c_sb = int(budgets[sb].sum()) // 128
                    mc0 = int(mask_seg_off[sb, 0]) // 128
                    f0 = int(cflat[mc0])
                    fcols = int(cflat[mc0 + c_sb]) - f0

                    mask_t = sp.tile([P, fcols], fp8, tag="mask")
                    ring = nc.sync if sb % 2 == 0 else nc.scalar
                    ring.dma_start(mask_t[:], mask_d[:, f0:f0 + fcols])

                    pa = psA.tile([P, SB_NODES], f32, tag="pa")
                    jj = 0
                    for b in range(N_BANKS):
                        gc0 = (int(seg_off[sb, b]) - gbase) // 128
                        for k in range(int(budgets[sb, b]) // 128):
                            mc = mc0 + jj
                            wk = int(cw[mc])
                            ok = int(coff[mc])
                            fk = int(cflat[mc]) - f0
                            nc.tensor.matmul(
                                out=pa[:, ok:ok + wk], lhsT=gat[:, gc0 + k, :],
                                rhs=mask_t[:, fk:fk + wk],
                                start=(jj == 0), stop=(jj == c_sb - 1))
                            jj += 1
                    aggs = vp.tile([P, SB_NODES], bf16, tag="aggs")
                    nc.vector.tensor_tensor(
                        out=aggs[:], in0=pa[:],
                        in1=recip_t[:, sb * SB_NODES:(sb + 1) * SB_NODES],
                        op=mybir.AluOpType.mult)

                    hpT = psM.tile([P, SB_NODES], f32, tag="hpT")
                    nc.tensor.matmul(out=hpT[:], lhsT=wl[:], rhs=aggs[:],
                                     start=True, stop=False)
                    nc.tensor.matmul(
                        out=hpT[:], lhsT=wr[:],
                        rhs=selfT[:, sb * SB_NODES:(sb + 1) * SB_NODES],
                        start=False, stop=True)

                    if rnd == 0:
                        # relu straight into the SBUF-resident h1selfT slice
                        nc.scalar.activation(
                            h1selfT_t[:, sb * SB_NODES:(sb + 1) * SB_NODES],
                            hpT[:], mybir.ActivationFunctionType.Relu,
                            bias=brow[:])
                        for t2 in range(2):
                            cols = slice(sb * SB_NODES + t2 * P,
                                         sb * SB_NODES + (t2 + 1) * P)
                            tp = psD.tile([P, P], bf16, tag="tp")
                            nc.tensor.transpose(
                                out=tp[:], in_=h1selfT_t[:, cols],
                                identity=consts["ident"][:])
                            h1row = vp.tile([P, P], bf16, tag="h1row")
                            nc.vector.tensor_copy(out=h1row[:], in_=tp[:])
                            nc.sync.dma_start(
                                h1tab_in[sb * SB_NODES + t2 * P:
                                         sb * SB_NODES + (t2 + 1) * P, :],
                                h1row[:])
                        if sb in AG_AFTER_SB:
                            emit_ag(AG_AFTER_SB[sb])
                    else:
                        h2T = vp.tile([P, SB_NODES], bf16, tag="h2T")
                        nc.scalar.activation(
                            h2T[:], hpT[:],
                            mybir.ActivationFunctionType.Copy)
                        d3 = []
                        for half in range(2):
                            dp = psD.tile([P, SB_NODES], f32, tag="dp")
                            nc.tensor.matmul(
                                out=dp[:],
                                lhsT=consts["W3T"][:, half * P:(half + 1) * P],
                                rhs=h2T[:], start=True, stop=True)
                            ds = vp.tile([P, SB_NODES], bf16, tag=f"d3{half}")
                            nc.scalar.activation(
                                ds[:], dp[:],
                                mybir.ActivationFunctionType.Relu,
                                bias=consts["b3c"][:, half:half + 1])
                            d3.append(ds)
                        op = psB.tile([1, SB_NODES], f32, tag="op")
                        nc.tensor.matmul(out=op[:], lhsT=consts["W4c"][:, 0:1],
                                         rhs=d3[0][:], start=True, stop=False)
                        nc.tensor.matmul(out=op[:], lhsT=consts["W4c"][:, 1:2],
                                         rhs=d3[1][:], start=False, stop=True)
                        orow = vp.tile([1, SB_NODES], f32, tag="orow")
                        nc.scalar.activation(
                            orow[:], op[:],
                            mybir.ActivationFunctionType.Copy,
                            bias=meta["b4"])
                        nc.scalar.dma_start(out_shard[sb:sb + 1, :], orow[:])

            emit_round(0, x_tab, consts["W1_lT"], consts["W1_rT"],
                       consts["b1c"])

            emit_round(1, h1tab, consts["W2_lT"], consts["W2_rT"],
                       consts["b1c"])

    nc.compile()
    return nc


def make_in_maps(meta):
    sm = meta["sm"]
    common = {"x_tab": meta["x_pad"],
              **{k: meta[k] for k in WNAMES}}
    maps = []
    for c in range(N_CORES):
        maps.append({
            **common,
            "x_selfT": meta["x_selfT"][c],
            "recip_bc": meta["recip_bc"][c],
            "idx": sm["idx"][c],
            "mask": sm["mask"][c],
        })
    return maps


_CACHE = {}


def _get_compiled(inputs, n_cores=8):
    assert n_cores == N_CORES
    meta = prep(inputs)
    key = (meta["sm"]["total_slots"],)
    if key not in _CACHE:
        _CACHE[key] = build(meta)
    return _CACHE[key], meta


def kernel(**inputs) -> np.ndarray:
    nc, meta = _get_compiled(inputs)
    in_maps = make_in_maps(meta)
    res = run_bass_kernel_spmd(nc, in_maps, core_ids=list(range(N_CORES)))
    out = np.empty(N_CORES * SHARD, dtype=np.float32)
    rl = _pad_local(np.arange(SHARD))
    for c in range(N_CORES):
        full = res.results[c]["out_shard"].reshape(-1)
        out[c * SHARD:(c + 1) * SHARD] = full[rl]
    return out



# revision 37
# speedup vs baseline: 1.0723x; 1.0723x over previous
"""GraphSAGE (2x SAGEConv mean-aggr + MLP decoder) on 8 Trainium2 NeuronCores.

v5 design (sim-trace driven):
- dst-node sharding, 12500/core padded to 12800 (4 quarters x 3200) so the
  h1 AllGather can run in 4 row-chunks overlapped with round-1 tail and
  round-2 bank-ordered gathers.
- Unified node numbering for both rounds: node v lives at row
  (v//12500)*12800 + quarter padding in BOTH the x table and the h1 table,
  so both rounds share identical gather indices and masks.
- WINDOWED one-hot masks: each 128-slot chunk (dst-sorted within one
  (sb,bank) segment) touches a ~56-wide dst window shared across cores, so
  the fp8 mask stores only [off_c, off_c+w_c) columns packed flat
  (12.6MB/round vs 57.5 full-width) and the PE matmul runs w_c cols
  instead of 256 (sim PE busy 432us -> 135us). The first chunk of every
  sb keeps the full 256 window and runs start=True, which doubles as the
  PSUM zero-init; later chunks accumulate into their column windows.
- Masks and idx split across the two HWDGE rings (SP + ACT) per parity.
- Gathers pair-granular (GS=2): one dma_gather per (sb-pair, bank) halves
  the SWDGE call count (200 vs 400) and idx DMA count; slot layout is
  pair-major/bank-major while masks stay sb-major flat-packed, paired in
  the matmul loop via host-computed chunk offsets.
- AllGather chunk q emitted mid-round-1 right after quarter q's h1tab_in
  rows are written, so the collective overlaps the round-1 tail on the
  in-order Pool queue.
- Mean: DVE multiply with host-replicated 1/deg; linears in T-orientation;
  round 1 relu -> SBUF-resident h1selfT -> PE transpose -> h1tab_in;
  decoder fused per superbatch, b2 folded into the decoder bias.
"""

import os

import numpy as np
import ml_dtypes

import concourse.bacc as bacc
import concourse.bass as bass
import concourse.mybir as mybir
import concourse.tile as tile
from concourse.bass_utils import run_bass_kernel_spmd
from concourse.library_config import mlp as mlp_lib

BF16 = ml_dtypes.bfloat16
FP8 = ml_dtypes.float8_e4m3fn

ABLATE = set(os.environ.get("K2_ABLATE", "").split(","))

N_CORES = 8
D = 128
P = 128
SB_NODES = 256
BANK = 25600
GATHER_CAP = 8192

SHARD = 12500
QCAP = 3200           # quarter capacity (multiple of 128)
SHARD_PAD = 4 * QCAP  # 12800
N_PAD = N_CORES * SHARD_PAD  # 102400
N_SB = SHARD_PAD // SB_NODES  # 50
N_BANKS = (N_PAD + BANK - 1) // BANK  # 4
GS = 2                # superbatches per gather group (one dma_gather per
                      # (group, bank) amortizes the ~1us SWDGE fixed cost)


def _pad_local(r):
    """local node index [0,12500) -> quarter-padded [0,12800)."""
    q = r // 3125
    return q * QCAP + (r - q * 3125)


def _unified_row(v):
    """global node id -> row in the unified padded table.

    Quarter-major: row = q*8*QCAP + core*QCAP + r_within_quarter, matching
    the layout the 4-chunk AllGather produces (chunk q = concat over cores
    of their quarter q), so AG chunk q fills exactly gather bank q."""
    c = v // SHARD
    r = v - c * SHARD
    q = r // 3125
    rq = r - q * 3125
    return q * (N_CORES * QCAP) + c * QCAP + rq


def _slot_meta(src_row, dst_pad, core_of_edge):
    """Group each core's edges by (sb, bank, dst); pad per-(sb,bank)
    segments to a common (max-over-cores, 128-aligned) budget.

    Returns per-core idx (int16 wrapped) + per-core fp8 one-hot masks
    [128, total_chunks, SB_NODES] + shared budgets/segment offsets."""
    sb = dst_pad // SB_NODES
    bank = src_row // BANK

    counts = np.zeros((N_CORES, N_SB, N_BANKS), dtype=np.int64)
    np.add.at(counts, (core_of_edge, sb, bank), 1)
    budgets = counts.max(axis=0)
    budgets = ((budgets + 127) // 128) * 128
    assert budgets.max() <= GATHER_CAP, budgets.max()

    # mask/slot-metadata order: sb-major (each sb's mask chunks contiguous)
    mask_seg_off = np.zeros((N_SB, N_BANKS), dtype=np.int64)
    flat = budgets.reshape(-1)
    mask_seg_off.reshape(-1)[1:] = np.cumsum(flat)[:-1]
    total_slots = int(flat.sum())
    n_chunks = total_slots // 128

    # gather order: pair-major, bank-major within pair (each (pair, bank)
    # is one contiguous dma_gather -> halves SWDGE call count)
    seg_off = np.zeros((N_SB, N_BANKS), dtype=np.int64)
    group_base = np.zeros(N_SB // GS + 1, dtype=np.int64)
    pos = 0
    for g in range(N_SB // GS):
        group_base[g] = pos
        for b in range(N_BANKS):
            for s in range(g * GS, (g + 1) * GS):
                seg_off[s, b] = pos
                pos += budgets[s, b]
    group_base[N_SB // GS] = pos
    assert pos == total_slots

    idx_cores, dw_cores = [], []
    for c in range(N_CORES):
        m = core_of_edge == c
        s_c, dp_c, sb_c, bk_c = (src_row[m], dst_pad[m], sb[m], bank[m])
        order = np.lexsort((dp_c, bk_c, sb_c))
        s_c, dp_c, sb_c, bk_c = (a[order] for a in (s_c, dp_c, sb_c, bk_c))

        idx_full = np.zeros(total_slots, dtype=np.int16)
        dstw_full = np.full(total_slots, -1, dtype=np.int64)
        cnt_c = np.zeros((N_SB, N_BANKS), dtype=np.int64)
        np.add.at(cnt_c, (sb_c, bk_c), 1)
        seg_start = np.zeros((N_SB, N_BANKS), dtype=np.int64)
        seg_start.reshape(-1)[1:] = np.cumsum(cnt_c.reshape(-1))[:-1]
        pos_in_seg = np.arange(len(s_c)) - seg_start[sb_c, bk_c]
        gslot = seg_off[sb_c, bk_c] + pos_in_seg
        mslot = mask_seg_off[sb_c, bk_c] + pos_in_seg
        idx_full[gslot] = (s_c - bk_c * BANK).astype(np.int16)
        dstw_full[mslot] = dp_c - sb_c * SB_NODES

        # idx wrap: slot i -> [i%16, i//16], replicated to 128 partitions
        w = idx_full.reshape(total_slots // 16, 16).T
        idx_cores.append(np.tile(w, (8, 1)).copy())
        dw_cores.append(dstw_full.reshape(n_chunks, 128))  # [chunks, 128]

    # Windowed masks: each chunk's slots (dst-sorted within one (sb,bank)
    # segment) touch a narrow dst window shared across cores. Store the
    # one-hot mask only over [off_c, off_c + w_c) columns, packed flat.
    # The FIRST chunk of every sb keeps the full SB_NODES window and runs
    # with start=True, which also zero-initializes the whole PSUM tile.
    all_dw = np.stack(dw_cores)                   # [cores, chunks, 128]
    vmask = all_dw >= 0
    lo = np.where(vmask, all_dw, SB_NODES).min(axis=(0, 2))   # [chunks]
    hi = np.where(vmask, all_dw, -1).max(axis=(0, 2))
    cw = np.where(hi >= lo, hi - lo + 1, 1)
    cw = np.minimum((cw + 15) // 16 * 16, SB_NODES)
    coff = np.minimum(np.maximum(lo, 0), SB_NODES - cw)
    chunk_sb = np.repeat(np.arange(N_SB), budgets.sum(axis=1) // 128)
    first = np.ones(n_chunks, dtype=bool)
    first[1:] = chunk_sb[1:] != chunk_sb[:-1]
    cw[first] = SB_NODES
    coff[first] = 0
    cflat = np.zeros(n_chunks + 1, dtype=np.int64)
    cflat[1:] = np.cumsum(cw)
    total_mask_cols = int(cflat[-1])

    mask_cores = []
    for c in range(N_CORES):
        dw = dw_cores[c]                          # [chunks, 128]
        mf = np.zeros((P, total_mask_cols), dtype=FP8)
        ch = np.repeat(np.arange(n_chunks), 128)
        pp = np.tile(np.arange(128), n_chunks)
        dwf = dw.reshape(-1)
        val = dwf >= 0
        mf[pp[val], cflat[ch[val]] + dwf[val] - coff[ch[val]]] = FP8(1.0)
        mask_cores.append(mf)

    return {
        "budgets": budgets, "seg_off": seg_off,
        "mask_seg_off": mask_seg_off, "group_base": group_base,
        "total_slots": total_slots,
        "n_chunks": n_chunks, "idx": idx_cores, "mask": mask_cores,
        "cw": cw, "coff": coff, "cflat": cflat,
        "total_mask_cols": total_mask_cols, "chunk_sb": chunk_sb,
    }


def prep(inputs):
    x = np.asarray(inputs["x"], dtype=np.float32)
    ei = np.asarray(inputs["edge_index"])
    n = x.shape[0]
    assert n == N_CORES * SHARD

    src = ei[0].astype(np.int64)
    dst = ei[1].astype(np.int64)
    src_row = _unified_row(src)
    core_of_edge = dst // SHARD
    dst_local = dst - core_of_edge * SHARD
    dst_pad = _pad_local(dst_local)

    sm = _slot_meta(src_row, dst_pad, core_of_edge)

    deg = np.bincount(dst, minlength=n).astype(np.float32)
    recip = (1.0 / np.maximum(deg, 1.0)).astype(np.float32)
    recip_pad = np.zeros((N_CORES, 1, SHARD_PAD), dtype=np.float32)
    x_selfT = np.zeros((N_CORES, D, SHARD_PAD), dtype=np.float32)
    for c in range(N_CORES):
        rl = _pad_local(np.arange(SHARD))
        recip_pad[c, 0, rl] = recip[c * SHARD:(c + 1) * SHARD]
        x_selfT[c, :, rl] = x[c * SHARD:(c + 1) * SHARD, :]

    x_pad = np.zeros((N_PAD, D), dtype=np.float32)
    x_pad[_unified_row(np.arange(n))] = x

    meta = {
        "sm": sm,
        "x_pad": x_pad.astype(BF16),
        "x_selfT": x_selfT.astype(BF16),
        "recip": recip_pad.astype(BF16),
        "W1_lT": np.asarray(inputs["W1_l"], np.float32).T.astype(BF16).copy(),
        "W1_rT": np.asarray(inputs["W1_r"], np.float32).T.astype(BF16).copy(),
        "W2_lT": np.asarray(inputs["W2_l"], np.float32).T.astype(BF16).copy(),
        "W2_rT": np.asarray(inputs["W2_r"], np.float32).T.astype(BF16).copy(),
        "W3T": np.asarray(inputs["W3"], np.float32).T.astype(BF16).copy(),
        "W4c": np.asarray(inputs["W4"], np.float32).reshape(2, 128).T
            .astype(BF16).copy(),
        "b1c": np.asarray(inputs["b1"], np.float32).reshape(-1, 1)
            .astype(BF16).copy(),
        # conv2 has no relu, so b2 folds exactly into the decoder bias:
        # relu(W3 @ (h2 + b2) + b3) = relu(W3 @ h2 + (b3 + W3 @ b2))
        "b3c": (np.asarray(inputs["b3"], np.float32)
                + np.asarray(inputs["W3"], np.float32)
                @ np.asarray(inputs["b2"], np.float32))
            .reshape(2, 128).T.astype(BF16).copy(),
        "b4": float(np.asarray(inputs["b4"]).reshape(-1)[0]),
        "ident": np.eye(P, dtype=BF16),
    }
    meta["recip_bc"] = np.broadcast_to(
        meta["recip"], (N_CORES, P, SHARD_PAD)).copy()
    return meta


WNAMES = ["W1_lT", "W1_rT", "W2_lT", "W2_rT", "W3T", "W4c",
          "b1c", "b3c", "ident"]


def build(meta):
    sm = meta["sm"]
    nc = bacc.Bacc("TRN2", target_bir_lowering=False, debug=False,
                   num_devices=N_CORES, num_swdge_queues=4)
    f32, bf16, fp8 = mybir.dt.float32, mybir.dt.bfloat16, mybir.dt.float8e4
    i16 = mybir.dt.int16

    x_tab = nc.dram_tensor("x_tab", [N_PAD, D], bf16, kind="ExternalInput")
    x_selfT_d = nc.dram_tensor("x_selfT", [D, SHARD_PAD], bf16,
                               kind="ExternalInput")
    recip_d = nc.dram_tensor("recip_bc", [P, SHARD_PAD], bf16,
                              kind="ExternalInput")
    idx_d = nc.dram_tensor("idx", list(sm["idx"][0].shape), i16,
                           kind="ExternalInput")
    mask_d = nc.dram_tensor("mask", [P, sm["total_mask_cols"]], fp8,
                            kind="ExternalInput")
    wt = {}
    for name in WNAMES:
        arr = meta[name]
        wt[name] = nc.dram_tensor(name, list(arr.shape), bf16,
                                  kind="ExternalInput")

    h1tab_in = nc.dram_tensor("h1tab_in", [SHARD_PAD, D], bf16)
    h1tab = nc.dram_tensor("h1tab", [N_PAD, D], bf16,
                           addr_space="Shared")
    out_shard = nc.dram_tensor("out_shard", [N_SB, SB_NODES], f32,
                               kind="ExternalOutput")

    budgets, seg_off = sm["budgets"], sm["seg_off"]
    mask_seg_off, group_base = sm["mask_seg_off"], sm["group_base"]
    cw, coff, cflat = sm["cw"], sm["coff"], sm["cflat"]

    with tile.TileContext(nc) as tc:
        with (
            tc.tile_pool(name="wp", bufs=1) as wp,
            tc.tile_pool(name="pp", bufs=3) as pp,
            tc.tile_pool(name="sp", bufs=6) as sp,
            tc.tile_pool(name="vp", bufs=4) as vp,
            tc.tile_pool(name="psA", bufs=2, space="PSUM") as psA,
            tc.tile_pool(name="psB", bufs=2, space="PSUM") as psB,
            tc.tile_pool(name="psM", bufs=2, space="PSUM") as psM,
            tc.tile_pool(name="psD", bufs=1, space="PSUM") as psD,
        ):
            with tc.tile_critical():
                nc.gpsimd.load_library(mlp_lib)

            consts = {}
            for name in WNAMES:
                t = wp.tile(list(meta[name].shape), bf16, tag=name)
                nc.sync.dma_start(t[:], wt[name][:])
                consts[name] = t
            recip_t = wp.tile([P, SHARD_PAD], bf16, tag="recip")
            nc.sync.dma_start(recip_t[:], recip_d[:])
            x_selfT_t = wp.tile([D, SHARD_PAD], bf16, tag="xselfT")
            nc.sync.dma_start(x_selfT_t[:], x_selfT_d[:])
            h1selfT_t = wp.tile([D, SHARD_PAD], bf16, tag="h1selfT")

            gq = [0]

            def emit_ag(q):
                if "ag" in ABLATE:
                    return
                nc.gpsimd.collective_compute(
                    "AllGather", mybir.AluOpType.bypass,
                    replica_groups=[list(range(N_CORES))],
                    ins=[h1tab_in[q * QCAP:(q + 1) * QCAP, :]],
                    outs=[h1tab[q * N_CORES * QCAP:
                                (q + 1) * N_CORES * QCAP, :]],
                )

            # emit AG chunk q as soon as the last h1tab_in row of quarter q
            # has been produced (quarter boundary 3200 = sb 12.5), so the
            # collective overlaps the round-1 tail on the in-order Pool queue
            AG_AFTER_SB = {12: 0, 24: 1, 37: 2, 49: 3}

            def emit_pair_gather(g, table):
                gbase = int(group_base[g])
                gslots = int(group_base[g + 1]) - gbase
                idx_t = pp.tile([P, gslots // 16], i16, tag="idx")
                iring = nc.scalar if g % 2 == 0 else nc.sync
                iring.dma_start(
                    idx_t[:],
                    idx_d[:, gbase // 16: (gbase + gslots) // 16])
                gat = pp.tile([P, gslots // 128, D], bf16, tag="gat")
                for b in range(N_BANKS):
                    nb = sum(int(budgets[s, b])
                             for s in range(g * GS, (g + 1) * GS))
                    if nb == 0:
                        continue
                    off = int(seg_off[g * GS, b]) - gbase
                    lo = b * BANK
                    hi = min(N_PAD, (b + 1) * BANK)
                    nc.gpsimd.dma_gather(
                        gat[:, off // 128: (off + nb) // 128, :],
                        table[lo:hi, :],
                        idx_t[:, off // 16: (off + nb) // 16],
                        num_idxs=nb, num_idxs_reg=nb, elem_size=D,
                        single_packet=False, queue_num=(b + g) % 4,
                    )
                    gq[0] += 1
                return gat, gbase

            def emit_round(rnd, table, wl, wr, brow):
                selfT = x_selfT_t if rnd == 0 else h1selfT_t
                for sb in range(N_SB):
                    if sb % GS == 0:
                        gat, gbase = emit_pair_gather(sb // GS, table)
                    c_sb = int(budgets[sb].sum()) // 128
                    mc0 = int(mask_seg_off[sb, 0]) // 128
                    f0 = int(cflat[mc0])
                    fcols = int(cflat[mc0 + c_sb]) - f0

                    mask_t = sp.tile([P, fcols], fp8, tag="mask")
                    ring = nc.sync if sb % 2 == 0 else nc.scalar
                    ring.dma_start(mask_t[:], mask_d[:, f0:f0 + fcols])

                    pa = psA.tile([P, SB_NODES], f32, tag="pa")
                    jj = 0
                    for b in range(N_BANKS):
                        gc0 = (int(seg_off[sb, b]) - gbase) // 128
                        for k in range(int(budgets[sb, b]) // 128):
                            mc = mc0 + jj
                            wk = int(cw[mc])
                            ok = int(coff[mc])
                            fk = int(cflat[mc]) - f0
                            nc.tensor.matmul(
                                out=pa[:, ok:ok + wk], lhsT=gat[:, gc0 + k, :],
                                rhs=mask_t[:, fk:fk + wk],
                                start=(jj == 0), stop=(jj == c_sb - 1))
                            jj += 1
                    aggs = vp.tile([P, SB_NODES], bf16, tag="aggs")
                    nc.vector.tensor_tensor(
                        out=aggs[:], in0=pa[:],
                        in1=recip_t[:, sb * SB_NODES:(sb + 1) * SB_NODES],
                        op=mybir.AluOpType.mult)

                    hpT = psM.tile([P, SB_NODES], f32, tag="hpT")
                    nc.tensor.matmul(out=hpT[:], lhsT=wl[:], rhs=aggs[:],
                                     start=True, stop=False)
                    nc.tensor.matmul(
                        out=hpT[:], lhsT=wr[:],
                        rhs=selfT[:, sb * SB_NODES:(sb + 1) * SB_NODES],
                        start=False, stop=True)

                    if rnd == 0:
                        # relu straight into the SBUF-resident h1selfT slice
                        nc.scalar.activation(
                            h1selfT_t[:, sb * SB_NODES:(sb + 1) * SB_NODES],
                            hpT[:], mybir.ActivationFunctionType.Relu,
                            bias=brow[:])
                        for t2 in range(2):
                            cols = slice(sb * SB_NODES + t2 * P,
                                         sb * SB_NODES + (t2 + 1) * P)
                            tp = psD.tile([P, P], bf16, tag="tp")
                            nc.tensor.transpose(
                                out=tp[:], in_=h1selfT_t[:, cols],
                                identity=consts["ident"][:])
                            h1row = vp.tile([P, P], bf16, tag="h1row")
                            nc.vector.tensor_copy(out=h1row[:], in_=tp[:])
                            nc.sync.dma_start(
                                h1tab_in[sb * SB_NODES + t2 * P:
                                         sb * SB_NODES + (t2 + 1) * P, :],
                                h1row[:])
                        if sb in AG_AFTER_SB:
                            emit_ag(AG_AFTER_SB[sb])
                    else:
                        h2T = vp.tile([P, SB_NODES], bf16, tag="h2T")
                        nc.scalar.activation(
                            h2T[:], hpT[:],
                            mybir.ActivationFunctionType.Copy)
                        d3 = []
                        for half in range(2):
                            dp = psD.tile([P, SB_NODES], f32, tag="dp")
                            nc.tensor.matmul(
                                out=dp[:],
                                lhsT=consts["W3T"][:, half * P:(half + 1) * P],
                                rhs=h2T[:], start=True, stop=True)
                            ds = vp.tile([P, SB_NODES], bf16, tag=f"d3{half}")
                            nc.scalar.activation(
                                ds[:], dp[:],
                                mybir.ActivationFunctionType.Relu,
                                bias=consts["b3c"][:, half:half + 1])
                            d3.append(ds)
                        op = psB.tile([1, SB_NODES], f32, tag="op")
                        nc.tensor.matmul(out=op[:], lhsT=consts["W4c"][:, 0:1],
                                         rhs=d3[0][:], start=True, stop=False)
                        nc.tensor.matmul(out=op[:], lhsT=consts["W4c"][:, 1:2],
                                         rhs=d3[1][:], start=False, stop=True)
                        orow = vp.tile([1, SB_NODES], f32, tag="orow")
                        nc.scalar.activation(
                            orow[:], op[:],
                            mybir.ActivationFunctionType.Copy,
                            bias=meta["b4"])
                        nc.scalar.dma_start(out_shard[sb:sb + 1, :], orow[:])

            emit_round(0, x_tab, consts["W1_lT"], consts["W1_rT"],
                       consts["b1c"])

            emit_round(1, h1tab, consts["W2_lT"], consts["W2_rT"],
                       consts["b1c"])

    nc.compile()
    return nc


def make_in_maps(meta):
    sm = meta["sm"]
    common = {"x_tab": meta["x_pad"],
              **{k: meta[k] for k in WNAMES}}
    maps = []
    for c in range(N_CORES):
        maps.append({
            **common,
            "x_selfT": meta["x_selfT"][c],
            "recip_bc": meta["recip_bc"][c],
            "idx": sm["idx"][c],
            "mask": sm["mask"][c],
        })
    return maps


_CACHE = {}


def _get_compiled(inputs, n_cores=8):
    assert n_cores == N_CORES
    meta = prep(inputs)
    key = (meta["sm"]["total_slots"],)
    if key not in _CACHE:
        _CACHE[key] = build(meta)
    return _CACHE[key], meta


def kernel(**inputs) -> np.ndarray:
    nc, meta = _get_compiled(inputs)
    in_maps = make_in_maps(meta)
    res = run_bass_kernel_spmd(nc, in_maps, core_ids=list(range(N_CORES)))
    out = np.empty(N_CORES * SHARD, dtype=np.float32)
    rl = _pad_local(np.arange(SHARD))
    for c in range(N_CORES):
        full = res.results[c]["out_shard"].reshape(-1)
        out[c * SHARD:(c + 1) * SHARD] = full[rl]
    return out

